# revision 5
# baseline (speedup 1.0000x reference)
"""Trainium2 Bass kernel for nn_ComplexNetMLP (complex ternary-quant MLP).

Strategy: data-parallel over the 8192 rows across 8 NeuronCores (1024 rows
each, no collectives). The ternary weight quantization and int8 activation
fake-quant mean every matmul is (small int) x {-1,0,+1}: exactly
representable in bf16 with exact fp32 PSUM accumulation, so the big
matmuls run at bf16 rate with integer-exact results. Per-row / global
scales are applied on the vector engines afterwards.

v2: 3-multiplication (Karatsuba) complex matmul for the up and down
projections: P = qxr@qwr, Q = qxi@qwi, R = (qxr-qxi)@(qwr+qwi), then
out_r = P+Q (exact) and out_i = R-P+Q. The combined weight qwr+qwi has
entries {0, +-rm, +-im} (disjoint ternary support), stored in bf16; the
difference activation D = qxr-qxi is built on-device in bf16. This cuts
tensor-engine work from 4 to 3 matmul units per complex linear. The gate
projection keeps 4 exact matmuls because its output feeds the relu2 sign
threshold, where bf16-sized errors in g_i cause mask flips worth ~2%
output error.

Host side only reformats weights: ternary sign matrices (bf16, pre
transposed so the contraction dim lands on SBUF partitions), combined
Karatsuba matrices for up/down, and global scale scalars.
"""

import sys

sys.path.insert(0, "/opt/trn_rl_repo")

import numpy as np
import ml_dtypes

import concourse.bass as bass
import concourse.tile as tile
from concourse import bacc, mybir
from concourse.bass_utils import run_bass_kernel_spmd

F32 = mybir.dt.float32
BF16 = mybir.dt.bfloat16

HIDDEN = 2048
IM = 5504
B, S = 4, 2048
ROWS = B * S              # 8192
NCORES = 8
RPC = ROWS // NCORES      # 1024 rows per core
HROWS = RPC // 2          # 512 rows per half
NRB = HROWS // 128        # 4 row-blocks per half
KO = HIDDEN // 128        # 16 k-chunks for gate/up
IO = IM // 128            # 43 im-chunks for down
EPS = 1e-6
MAGIC = float(np.float32(12582912.0))  # 1.5 * 2**23, RNE rounding magic

# im tiles for phase 1 (gate/up output tiles)
P1_W = 256
IM_TILES = [(i * P1_W, min(P1_W, IM - i * P1_W)) for i in range((IM + P1_W - 1) // P1_W)]
HT_W = 256                 # hidden tile width for down proj
HID_TILES = [(i * HT_W, HT_W) for i in range(HIDDEN // HT_W)]
# phase-2 requant chunks: (io0, nio) covering IO=43
P2_CHUNKS = [(0, 22), (22, 21)]
P2_WMAX = 22 * 128

Alu = mybir.AluOpType
Act = mybir.ActivationFunctionType


def _weight_prep(wr, wi):
    """Replicate reference.weight_quant: ternary sign matrices + scale.

    Returns (tr, ti) sign matrices in {-1,0,1} and (rmean, imean)
    clipped scales so qwr = tr * rm, qwi = ti * im.
    Masks via |wr| vs |wi| (equivalent to the phase comparisons except on
    measure-zero boundaries).
    """
    awr = np.abs(wr)
    awi = np.abs(wi)
    rmask = awr > awi
    imask = ~rmask
    tr = np.where(rmask, np.sign(wr), 0.0).astype(np.float32)
    ti = np.where(imask, np.sign(wi), 0.0).astype(np.float32)
    rcnt = np.maximum(rmask.sum(dtype=np.float32), 1.0)
    icnt = np.maximum(imask.sum(dtype=np.float32), 1.0)
    rmean = np.float32(np.sum(awr * rmask, dtype=np.float32) / rcnt)
    imean = np.float32(np.sum(awi * imask, dtype=np.float32) / icnt)
    rm = float(np.clip(rmean, np.float32(1e-5), None))
    im = float(np.clip(imean, np.float32(1e-5), None))
    return tr, ti, rm, im


def _to_bf16_T(t):
    """Transpose and cast a weight matrix to contiguous bf16."""
    return np.ascontiguousarray(t.T).astype(ml_dtypes.bfloat16)


def _build_program(rm_g, im_g, rm_u, im_u, rm_d, im_d, rpc=RPC):
    """Trace the per-core Bass program (same program for all 8 cores)."""
    HROWS = rpc // 2
    NRB = HROWS // 128
    nc = bacc.Bacc("TRN2", target_bir_lowering=False, debug=False,
                   num_devices=NCORES)

    xr_d = nc.dram_tensor("xr", [rpc, HIDDEN], F32, kind="ExternalInput")
    xi_d = nc.dram_tensor("xi", [rpc, HIDDEN], F32, kind="ExternalInput")
    wgr_d = nc.dram_tensor("wgr_t", [HIDDEN, IM], BF16, kind="ExternalInput")
    wgi_d = nc.dram_tensor("wgi_t", [HIDDEN, IM], BF16, kind="ExternalInput")
    wur_d = nc.dram_tensor("wur_t", [HIDDEN, IM], BF16, kind="ExternalInput")
    wui_d = nc.dram_tensor("wui_t", [HIDDEN, IM], BF16, kind="ExternalInput")
    wuc_d = nc.dram_tensor("wuc_t", [HIDDEN, IM], BF16, kind="ExternalInput")
    wdr_d = nc.dram_tensor("wdr_t", [IM, HIDDEN], BF16, kind="ExternalInput")
    wdi_d = nc.dram_tensor("wdi_t", [IM, HIDDEN], BF16, kind="ExternalInput")
    wdc_d = nc.dram_tensor("wdc_t", [IM, HIDDEN], BF16, kind="ExternalInput")
    lnr_d = nc.dram_tensor("ln_r", [1, IM], F32, kind="ExternalInput")
    lni_d = nc.dram_tensor("ln_i", [1, IM], F32, kind="ExternalInput")
    or_d = nc.dram_tensor("o_r", [rpc, HIDDEN], F32, kind="ExternalOutput")
    oi_d = nc.dram_tensor("o_i", [rpc, HIDDEN], F32, kind="ExternalOutput")

    tr_d = nc.dram_tensor("t_r_scratch", [rpc, IM], F32)
    ti_d = nc.dram_tensor("t_i_scratch", [rpc, IM], F32)

    # weight DRAM views with the 128-partition k-chunk factored out
    wg_views = {
        "gr": wgr_d.ap().rearrange("(ko ki) n -> ki ko n", ki=128),
        "gi": wgi_d.ap().rearrange("(ko ki) n -> ki ko n", ki=128),
        "ur": wur_d.ap().rearrange("(ko ki) n -> ki ko n", ki=128),
        "ui": wui_d.ap().rearrange("(ko ki) n -> ki ko n", ki=128),
        "uc": wuc_d.ap().rearrange("(ko ki) n -> ki ko n", ki=128),
    }
    wd_views = {
        "dr": wdr_d.ap().rearrange("(io ii) n -> ii io n", ii=128),
        "di": wdi_d.ap().rearrange("(io ii) n -> ii io n", ii=128),
        "dc": wdc_d.ap().rearrange("(io ii) n -> ii io n", ii=128),
    }

    def ln_bcast(handle, n0, w):
        return bass.AP(tensor=handle, offset=n0, ap=[[0, 128], [1, w]])

    with tile.TileContext(nc) as tc:
        for half in range(2):
            r0 = half * HROWS

            with tc.tile_pool(name=f"stats{half}", bufs=1) as stats:
              with tc.tile_pool(name=f"xqt{half}", bufs=1) as xqt_pool:

                # ---- prologue: act quant of x + D = qxr - qxi + transposes ----
                xqt = {"r": [], "i": []}
                dxt = []                     # transposed D tiles per rb
                mclip = {"r": [], "i": []}   # clip(absmax, 1e-5) per rb
                alpha = []                   # per rb dict of matmul scales
                for rb in range(NRB):
                    rr = r0 + rb * 128
                    al = {}
                    with tc.tile_pool(name=f"prol{half}_{rb}", bufs=1) as pp:
                        xq = {}
                        for comp, x_d in (("r", xr_d), ("i", xi_d)):
                            xt = pp.tile([128, HIDDEN], F32,
                                         name=f"x{comp}{half}{rb}")
                            nc.sync.dma_start(xt[:], x_d.ap()[rr:rr + 128, :])
                            am = stats.tile([128, 1], F32,
                                            name=f"am{comp}{half}{rb}")
                            nc.vector.tensor_reduce(
                                out=am[:], in_=xt[:], axis=mybir.AxisListType.X,
                                op=Alu.max, apply_absolute_value=True)
                            m = stats.tile([128, 1], F32,
                                           name=f"m{comp}{half}{rb}")
                            nc.vector.tensor_scalar_max(out=m[:], in0=am[:],
                                                        scalar1=1e-5)
                            mclip[comp].append(m)
                            rcp = stats.tile([128, 1], F32,
                                             name=f"rcp{comp}{half}{rb}")
                            nc.vector.reciprocal(rcp[:], m[:])
                            c = stats.tile([128, 1], F32,
                                           name=f"c{comp}{half}{rb}")
                            nc.vector.tensor_scalar_mul(out=c[:], in0=rcp[:],
                                                        scalar1=127.0)
                            s1 = pp.tile([128, HIDDEN], F32,
                                         name=f"s1{comp}{half}{rb}")
                            nc.vector.tensor_scalar(
                                out=s1[:], in0=xt[:], scalar1=c[:],
                                scalar2=MAGIC, op0=Alu.mult, op1=Alu.add)
                            xqc = pp.tile([128, HIDDEN], BF16,
                                          name=f"xq{comp}{half}{rb}")
                            nc.vector.tensor_scalar(
                                out=xqc[:], in0=s1[:], scalar1=MAGIC,
                                scalar2=None, op0=Alu.subtract, op1=Alu.bypass)
                            xq[comp] = xqc
                            xqt_t = xqt_pool.tile([128, KO, 128], BF16,
                                                  name=f"xqt{comp}{half}{rb}")
                            nc.sync.dma_start_transpose(xqt_t[:], xqc[:])
                            xqt[comp].append(xqt_t)
                        # D = qxr - qxi = xq_r*(mr/127) - xq_i*(mi/127)
                        sr5 = stats.tile([128, 1], F32, name=f"sr5{half}{rb}")
                        nc.vector.tensor_scalar_mul(
                            out=sr5[:], in0=mclip["r"][rb][:], scalar1=1.0 / 127.0)
                        si5n = stats.tile([128, 1], F32, name=f"si5n{half}{rb}")
                        nc.vector.tensor_scalar_mul(
                            out=si5n[:], in0=mclip["i"][rb][:], scalar1=-1.0 / 127.0)
                        dtmp = pp.tile([128, HIDDEN], F32,
                                       name=f"dtmp{half}{rb}")
                        nc.scalar.activation(dtmp[:], xq["r"][:], Act.Copy,
                                             scale=sr5[:])
                        dbf = pp.tile([128, HIDDEN], BF16,
                                      name=f"dbf{half}{rb}")
                        nc.vector.scalar_tensor_tensor(
                            out=dbf[:], in0=xq["i"][:], scalar=si5n[:],
                            in1=dtmp[:], op0=Alu.mult, op1=Alu.add)
                        dxt_t = xqt_pool.tile([128, KO, 128], BF16,
                                              name=f"dxt{half}{rb}")
                        nc.sync.dma_start_transpose(dxt_t[:], dbf[:])
                        dxt.append(dxt_t)
                    # per-row matmul output scales
                    for nm, const, m in (
                        ("g1", rm_g, mclip["r"][rb]), ("g2", im_g, mclip["i"][rb]),
                        ("g3", im_g, mclip["r"][rb]), ("g4", rm_g, mclip["i"][rb]),
                        ("u1", rm_u, mclip["r"][rb]), ("u2", im_u, mclip["i"][rb]),
                        ("u1n", -rm_u, mclip["r"][rb]),
                    ):
                        t = stats.tile([128, 1], F32, name=f"al{nm}{half}{rb}")
                        nc.vector.tensor_scalar_mul(out=t[:], in0=m[:],
                                                    scalar1=const / 127.0)
                        al[nm] = t
                    alpha.append(al)

                # accumulators (ping-pong) per rb: sumsq r/i, absmax r/i
                acc = {k: [[stats.tile([128, 1], F32, name=f"acc{k}{half}{rb}{s}")
                            for s in range(2)] for rb in range(NRB)]
                       for k in ("ssr", "ssi", "amr", "ami")}

                # ---- phase 1: gate + up + relu2 + glu + stats ----
                with tc.tile_pool(name=f"wg{half}", bufs=2) as wpool, \
                     tc.tile_pool(name=f"ln{half}", bufs=2) as lnpool, \
                     tc.tile_pool(name=f"gsto{half}", bufs=2) as gpool, \
                     tc.tile_pool(name=f"tout{half}", bufs=3) as tpool, \
                     tc.tile_pool(name=f"scr{half}", bufs=2) as spool, \
                     tc.tile_pool(name=f"ps{half}", bufs=1, space="PSUM") as psp:
                    for it, (n0, w) in enumerate(IM_TILES):
                        wt = {}
                        for key in ("gr", "gi", "ur", "ui", "uc"):
                            t = wpool.tile([128, KO, P1_W], BF16, tag=f"w{key}",
                                           name=f"w{key}_{half}_{it}")
                            for ko in range(KO):
                                nc.sync.dma_start(t[:, ko, :w],
                                                  wg_views[key][:, ko, n0:n0 + w])
                            wt[key] = t
                        lnr_t = lnpool.tile([128, P1_W], F32, tag="lnr",
                                            name=f"lnr{half}_{it}")
                        nc.gpsimd.dma_start(out=lnr_t[:, :w],
                                            in_=ln_bcast(lnr_d, n0, w))
                        lni_t = lnpool.tile([128, P1_W], F32, tag="lni",
                                            name=f"lni{half}_{it}")
                        nc.gpsimd.dma_start(out=lni_t[:, :w],
                                            in_=ln_bcast(lni_d, n0, w))

                        for rb in range(NRB):
                            rr = r0 + rb * 128
                            ps = {k: psp.tile([128, P1_W], F32, tag=f"ps{k}",
                                              name=f"ps{k}_{half}_{it}_{rb}")
                                  for k in ("g1", "g2", "g3", "g4",
                                            "u1", "u2", "uR")}
                            for ko in range(KO):
                                st = ko == 0
                                sp = ko == KO - 1
                                lr = xqt["r"][rb][:, ko, :]
                                li = xqt["i"][rb][:, ko, :]
                                ld = dxt[rb][:, ko, :]
                                nc.tensor.matmul(ps["g1"][:, :w], lr,
                                                 wt["gr"][:, ko, :w],
                                                 start=st, stop=sp)
                                nc.tensor.matmul(ps["g3"][:, :w], lr,
                                                 wt["gi"][:, ko, :w],
                                                 start=st, stop=sp)
                                nc.tensor.matmul(ps["u1"][:, :w], lr,
                                                 wt["ur"][:, ko, :w],
                                                 start=st, stop=sp)
                                nc.tensor.matmul(ps["g2"][:, :w], li,
                                                 wt["gi"][:, ko, :w],
                                                 start=st, stop=sp)
                                nc.tensor.matmul(ps["g4"][:, :w], li,
                                                 wt["gr"][:, ko, :w],
                                                 start=st, stop=sp)
                                nc.tensor.matmul(ps["u2"][:, :w], li,
                                                 wt["ui"][:, ko, :w],
                                                 start=st, stop=sp)
                                nc.tensor.matmul(ps["uR"][:, :w], ld,
                                                 wt["uc"][:, ko, :w],
                                                 start=st, stop=sp)

                            al = alpha[rb]
                            # g_r = a1*G1 + a2*G2 ; g_i = a3*G3 - a4*G4
                            tb = spool.tile([128, P1_W], F32, tag="tb",
                                            name=f"tb{half}_{it}_{rb}")
                            nc.scalar.activation(tb[:, :w], ps["g2"][:, :w],
                                                 Act.Copy, scale=al["g2"][:])
                            g_r = gpool.tile([128, P1_W], F32, tag="g_r",
                                             name=f"g_r{half}_{it}_{rb}")
                            nc.vector.scalar_tensor_tensor(
                                out=g_r[:, :w], in0=ps["g1"][:, :w],
                                scalar=al["g1"][:], in1=tb[:, :w],
                                op0=Alu.mult, op1=Alu.add)
                            tb2 = spool.tile([128, P1_W], F32, tag="tb2",
                                             name=f"tb2{half}_{it}_{rb}")
                            nc.scalar.activation(tb2[:, :w], ps["g4"][:, :w],
                                                 Act.Copy, scale=al["g4"][:])
                            g_i = gpool.tile([128, P1_W], F32, tag="g_i",
                                             name=f"g_i{half}_{it}_{rb}")
                            nc.vector.scalar_tensor_tensor(
                                out=g_i[:, :w], in0=ps["g3"][:, :w],
                                scalar=al["g3"][:], in1=tb2[:, :w],
                                op0=Alu.mult, op1=Alu.subtract)
                            # u_r = b1*U1 + b2*U2 ; u_i = R - b1*U1 + b2*U2
                            tb3 = spool.tile([128, P1_W], F32, tag="tb3",
                                             name=f"tb3{half}_{it}_{rb}")
                            nc.scalar.activation(tb3[:, :w], ps["u2"][:, :w],
                                                 Act.Copy, scale=al["u2"][:])
                            u_r = gpool.tile([128, P1_W], F32, tag="u_r",
                                             name=f"u_r{half}_{it}_{rb}")
                            nc.vector.scalar_tensor_tensor(
                                out=u_r[:, :w], in0=ps["u1"][:, :w],
                                scalar=al["u1"][:], in1=tb3[:, :w],
                                op0=Alu.mult, op1=Alu.add)
                            tb4 = spool.tile([128, P1_W], F32, tag="tb4",
                                             name=f"tb4{half}_{it}_{rb}")
                            nc.vector.tensor_tensor(
                                out=tb4[:, :w], in0=ps["uR"][:, :w],
                                in1=tb3[:, :w], op=Alu.add)
                            u_i = gpool.tile([128, P1_W], F32, tag="u_i",
                                             name=f"u_i{half}_{it}_{rb}")
                            nc.vector.scalar_tensor_tensor(
                                out=u_i[:, :w], in0=ps["u1"][:, :w],
                                scalar=al["u1n"][:], in1=tb4[:, :w],
                                op0=Alu.mult, op1=Alu.add)

                            # relu2 keep-mask c = (max(g_r, g_i) >= 0)
                            cm = spool.tile([128, P1_W], F32, tag="cm",
                                            name=f"cm{half}_{it}_{rb}")
                            nc.vector.tensor_max(out=cm[:, :w], in0=g_r[:, :w],
                                                 in1=g_i[:, :w])
                            nc.vector.tensor_scalar(
                                out=cm[:, :w], in0=cm[:, :w], scalar1=0.0,
                                scalar2=None, op0=Alu.is_ge, op1=Alu.bypass)
                            p2 = spool.tile([128, P1_W], F32, tag="p2",
                                            name=f"p2{half}_{it}_{rb}")
                            nc.scalar.activation(p2[:, :w], g_r[:, :w],
                                                 Act.Square)
                            q2 = spool.tile([128, P1_W], F32, tag="q2",
                                            name=f"q2{half}_{it}_{rb}")
                            nc.scalar.activation(q2[:, :w], g_i[:, :w],
                                                 Act.Square)
                            # h_r = (p2*u_r + q2*u_i) * c
                            t1 = spool.tile([128, P1_W], F32, tag="t1",
                                            name=f"t1{half}_{it}_{rb}")
                            nc.vector.tensor_mul(out=t1[:, :w], in0=p2[:, :w],
                                                 in1=u_r[:, :w])
                            t2 = spool.tile([128, P1_W], F32, tag="t2",
                                            name=f"t2{half}_{it}_{rb}")
                            nc.vector.tensor_mul(out=t2[:, :w], in0=q2[:, :w],
                                                 in1=u_i[:, :w])
                            h_r = spool.tile([128, P1_W], F32, tag="h_r",
                                             name=f"h_r{half}_{it}_{rb}")
                            nc.vector.tensor_add(out=h_r[:, :w], in0=t1[:, :w],
                                                 in1=t2[:, :w])
                            nc.vector.tensor_mul(out=h_r[:, :w], in0=h_r[:, :w],
                                                 in1=cm[:, :w])
                            # h_i = (p2*u_i - q2*u_r) * c
                            t3 = spool.tile([128, P1_W], F32, tag="t3",
                                            name=f"t3{half}_{it}_{rb}")
                            nc.vector.tensor_mul(out=t3[:, :w], in0=p2[:, :w],
                                                 in1=u_i[:, :w])
                            t4 = spool.tile([128, P1_W], F32, tag="t4",
                                            name=f"t4{half}_{it}_{rb}")
                            nc.vector.tensor_mul(out=t4[:, :w], in0=q2[:, :w],
                                                 in1=u_r[:, :w])
                            h_i = spool.tile([128, P1_W], F32, tag="h_i",
                                             name=f"h_i{half}_{it}_{rb}")
                            nc.vector.tensor_sub(out=h_i[:, :w], in0=t3[:, :w],
                                                 in1=t4[:, :w])
                            nc.vector.tensor_mul(out=h_i[:, :w], in0=h_i[:, :w],
                                                 in1=cm[:, :w])

                            # stats + ln-scaled spill tiles
                            sq = spool.tile([128, P1_W], F32, tag="sq",
                                            name=f"sq{half}_{it}_{rb}")
                            nc.vector.tensor_mul(out=sq[:, :w], in0=h_r[:, :w],
                                                 in1=h_r[:, :w])
                            ssp = spool.tile([128, 1], F32, tag="ssp",
                                             name=f"ssp{half}_{it}_{rb}")
                            nc.vector.tensor_reduce(
                                out=ssp[:], in_=sq[:, :w], axis=mybir.AxisListType.X,
                                op=Alu.add)
                            if it == 0:
                                nc.vector.tensor_copy(out=acc["ssr"][rb][0][:],
                                                      in_=ssp[:])
                            else:
                                nc.vector.tensor_add(
                                    out=acc["ssr"][rb][it % 2][:],
                                    in0=acc["ssr"][rb][(it - 1) % 2][:], in1=ssp[:])
                            t_r = tpool.tile([128, P1_W], F32, tag="t_r",
                                             name=f"t_r{half}_{it}_{rb}")
                            nc.vector.tensor_mul(out=t_r[:, :w], in0=h_r[:, :w],
                                                 in1=lnr_t[:, :w])
                            amp = spool.tile([128, 1], F32, tag="amp",
                                             name=f"amp{half}_{it}_{rb}")
                            nc.vector.tensor_reduce(
                                out=amp[:], in_=t_r[:, :w], axis=mybir.AxisListType.X,
                                op=Alu.max, apply_absolute_value=True)
                            if it == 0:
                                nc.vector.tensor_scalar_max(
                                    out=acc["amr"][rb][0][:], in0=amp[:], scalar1=0.0)
                            else:
                                nc.vector.tensor_tensor(
                                    out=acc["amr"][rb][it % 2][:],
                                    in0=acc["amr"][rb][(it - 1) % 2][:],
                                    in1=amp[:], op=Alu.max)
                            sq2 = spool.tile([128, P1_W], F32, tag="sq2",
                                             name=f"sq2{half}_{it}_{rb}")
                            nc.vector.tensor_mul(out=sq2[:, :w], in0=h_i[:, :w],
                                                 in1=h_i[:, :w])
                            ssp2 = spool.tile([128, 1], F32, tag="ssp2",
                                              name=f"ssp2{half}_{it}_{rb}")
                            nc.vector.tensor_reduce(
                                out=ssp2[:], in_=sq2[:, :w], axis=mybir.AxisListType.X,
                                op=Alu.add)
                            if it == 0:
                                nc.vector.tensor_copy(out=acc["ssi"][rb][0][:],
                                                      in_=ssp2[:])
                            else:
                                nc.vector.tensor_add(
                                    out=acc["ssi"][rb][it % 2][:],
                                    in0=acc["ssi"][rb][(it - 1) % 2][:], in1=ssp2[:])
                            t_i = tpool.tile([128, P1_W], F32, tag="t_i",
                                             name=f"t_i{half}_{it}_{rb}")
                            nc.vector.tensor_mul(out=t_i[:, :w], in0=h_i[:, :w],
                                                 in1=lni_t[:, :w])
                            amp2 = spool.tile([128, 1], F32, tag="amp2",
                                              name=f"amp2{half}_{it}_{rb}")
                            nc.vector.tensor_reduce(
                                out=amp2[:], in_=t_i[:, :w], axis=mybir.AxisListType.X,
                                op=Alu.max, apply_absolute_value=True)
                            if it == 0:
                                nc.vector.tensor_scalar_max(
                                    out=acc["ami"][rb][0][:], in0=amp2[:], scalar1=0.0)
                            else:
                                nc.vector.tensor_tensor(
                                    out=acc["ami"][rb][it % 2][:],
                                    in0=acc["ami"][rb][(it - 1) % 2][:],
                                    in1=amp2[:], op=Alu.max)
                            nc.sync.dma_start(tr_d.ap()[rr:rr + 128, n0:n0 + w],
                                              t_r[:, :w])
                            nc.sync.dma_start(ti_d.ap()[rr:rr + 128, n0:n0 + w],
                                              t_i[:, :w])

              # ---- phase 2: rmsnorm scale + act quant of n + Dn + transposes ----
              last = (len(IM_TILES) - 1) % 2
              with tc.tile_pool(name=f"nqt{half}", bufs=1) as nqt_pool:
                  nqt = {"r": [], "i": []}
                  dnt = []
                  dsc = []
                  with tc.tile_pool(name=f"p2a{half}", bufs=2) as p2a, \
                       tc.tile_pool(name=f"p2s{half}", bufs=1) as p2s:
                      for rb in range(NRB):
                          rr = r0 + rb * 128
                          ssr = acc["ssr"][rb][last]
                          ssi = acc["ssi"][rb][last]
                          su = stats.tile([128, 1], F32, name=f"su{half}{rb}")
                          nc.vector.tensor_add(out=su[:], in0=ssr[:], in1=ssi[:])
                          me = stats.tile([128, 1], F32, name=f"me{half}{rb}")
                          nc.vector.tensor_scalar(
                              out=me[:], in0=su[:], scalar1=1.0 / IM,
                              scalar2=EPS, op0=Alu.mult, op1=Alu.add)
                          sr = stats.tile([128, 1], F32, name=f"sr{half}{rb}")
                          nc.scalar.activation(sr[:], me[:], Act.Sqrt)
                          inv0 = stats.tile([128, 1], F32, name=f"inv0{half}{rb}")
                          nc.vector.reciprocal(inv0[:], sr[:])
                          # one Newton step: inv = inv0*(1.5 - 0.5*me*inv0^2)
                          nw = stats.tile([128, 1], F32, name=f"nw{half}{rb}")
                          nc.vector.tensor_mul(out=nw[:], in0=inv0[:], in1=inv0[:])
                          nc.vector.tensor_mul(out=nw[:], in0=nw[:], in1=me[:])
                          nc.vector.tensor_scalar(
                              out=nw[:], in0=nw[:], scalar1=-0.5, scalar2=1.5,
                              op0=Alu.mult, op1=Alu.add)
                          inv = stats.tile([128, 1], F32, name=f"inv{half}{rb}")
                          nc.vector.tensor_mul(out=inv[:], in0=inv0[:], in1=nw[:])

                          sc = {}
                          for comp, amk in (("r", "amr"), ("i", "ami")):
                              am = acc[amk][rb][last]
                              amn = stats.tile([128, 1], F32,
                                               name=f"amn{comp}{half}{rb}")
                              nc.vector.tensor_mul(out=amn[:], in0=am[:], in1=inv[:])
                              nc.vector.tensor_scalar_max(out=amn[:], in0=amn[:],
                                                          scalar1=1e-5)
                              rsn = stats.tile([128, 1], F32,
                                               name=f"rsn{comp}{half}{rb}")
                              nc.vector.reciprocal(rsn[:], amn[:])
                              nc.vector.tensor_scalar_mul(out=rsn[:], in0=rsn[:],
                                                          scalar1=127.0)
                              cq = stats.tile([128, 1], F32,
                                              name=f"cq{comp}{half}{rb}")
                              nc.vector.tensor_mul(out=cq[:], in0=inv[:], in1=rsn[:])
                              sc[f"cq{comp}"] = cq
                              sc[f"amn{comp}"] = amn
                          # down-proj combine scales (Karatsuba)
                          for nm, const, amn in (
                              ("d1", rm_d, sc["amnr"]), ("d2", im_d, sc["amni"]),
                              ("d1n", -rm_d, sc["amnr"]),
                          ):
                              t = stats.tile([128, 1], F32,
                                             name=f"ds{nm}{half}{rb}")
                              nc.vector.tensor_scalar_mul(out=t[:], in0=amn[:],
                                                          scalar1=const / 127.0)
                              sc[nm] = t
                          # Dn = nq_r*(amnr/127) - nq_i*(amni/127)
                          cr5 = stats.tile([128, 1], F32, name=f"cr5{half}{rb}")
                          nc.vector.tensor_scalar_mul(
                              out=cr5[:], in0=sc["amnr"][:], scalar1=1.0 / 127.0)
                          ci5n = stats.tile([128, 1], F32, name=f"ci5n{half}{rb}")
                          nc.vector.tensor_scalar_mul(
                              out=ci5n[:], in0=sc["amni"][:], scalar1=-1.0 / 127.0)
                          dsc.append(sc)

                          nqt_t = {}
                          for comp in ("r", "i"):
                              nqt_t[comp] = nqt_pool.tile(
                                  [128, IO, 128], BF16, name=f"nqt{comp}{half}{rb}")
                              nqt[comp].append(nqt_t[comp])
                          dnt_t = nqt_pool.tile([128, IO, 128], BF16,
                                                name=f"dnt{half}{rb}")
                          dnt.append(dnt_t)

                          for ci, (io0, nio) in enumerate(P2_CHUNKS):
                              c0 = io0 * 128
                              w = nio * 128
                              nq = {}
                              for comp, t_d in (("r", tr_d), ("i", ti_d)):
                                  tin = p2a.tile([128, P2_WMAX], F32, tag="tin",
                                                 name=f"tin{comp}{half}{rb}{ci}")
                                  nc.sync.dma_start(
                                      tin[:, :w], t_d.ap()[rr:rr + 128, c0:c0 + w])
                                  s1 = p2s.tile([128, P2_WMAX], F32, tag="s1q",
                                                name=f"s1q{comp}{half}{rb}{ci}")
                                  nc.scalar.activation(s1[:, :w], tin[:, :w],
                                                       Act.Copy, bias=MAGIC,
                                                       scale=sc[f"cq{comp}"][:])
                                  nqc = p2s.tile([128, P2_WMAX], BF16,
                                                 tag=f"nq{comp}",
                                                 name=f"nq{comp}{half}{rb}{ci}")
                                  nc.vector.tensor_scalar(
                                      out=nqc[:, :w], in0=s1[:, :w], scalar1=MAGIC,
                                      scalar2=None, op0=Alu.subtract, op1=Alu.bypass)
                                  nq[comp] = nqc
                                  nc.sync.dma_start_transpose(
                                      nqt_t[comp][:, io0:io0 + nio, :], nqc[:, :w])
                              dtm = p2s.tile([128, P2_WMAX], F32, tag="dtmp",
                                             name=f"dtm{half}{rb}{ci}")
                              nc.scalar.activation(dtm[:, :w], nq["r"][:, :w],
                                                   Act.Copy, scale=cr5[:])
                              dnc = p2s.tile([128, P2_WMAX], BF16, tag="dn",
                                             name=f"dnc{half}{rb}{ci}")
                              nc.vector.scalar_tensor_tensor(
                                  out=dnc[:, :w], in0=nq["i"][:, :w],
                                  scalar=ci5n[:], in1=dtm[:, :w],
                                  op0=Alu.mult, op1=Alu.add)
                              nc.sync.dma_start_transpose(
                                  dnt_t[:, io0:io0 + nio, :], dnc[:, :w])

                  # ---- phase 3: down projection (Karatsuba) ----
                  with tc.tile_pool(name=f"dt{half}", bufs=2) as dtpool, \
                       tc.tile_pool(name=f"oo{half}", bufs=3) as opool, \
                       tc.tile_pool(name=f"od{half}", bufs=2) as ospool, \
                       tc.tile_pool(name=f"psd{half}", bufs=2,
                                    space="PSUM") as psd:
                      # Incremental combine keeps only one PSUM bank per rb
                      # live at a time:
                      #   after P:  t1 = d1*P            (pd bank freed)
                      #   after Q:  ob = d2*Q; o_r = t1+ob; t2 = ob-t1
                      #   after R:  o_i = R + t2
                      for ht, (h0, hw) in enumerate(HID_TILES):
                          t1 = {}
                          t2 = {}
                          for mat, key, stat in (("r", "dr", nqt["r"]),
                                                 ("i", "di", nqt["i"]),
                                                 ("c", "dc", dnt)):
                              wtd = dtpool.tile([128, IO, HT_W], BF16, tag="wd",
                                                name=f"wd{mat}{half}_{ht}")
                              for io in range(IO):
                                  nc.sync.dma_start(wtd[:, io, :],
                                                    wd_views[key][:, io, h0:h0 + hw])
                              for rb in range(NRB):
                                  rr = r0 + rb * 128
                                  sc = dsc[rb]
                                  pdt = psd.tile([128, HT_W], F32,
                                                 tag=f"pd{rb}",
                                                 name=f"pd{mat}{rb}_{half}_{ht}")
                                  for io in range(IO):
                                      nc.tensor.matmul(pdt[:], stat[rb][:, io, :],
                                                       wtd[:, io, :],
                                                       start=io == 0,
                                                       stop=io == IO - 1)
                                  if mat == "r":
                                      t1t = ospool.tile([128, HT_W], F32,
                                                        tag=f"t1_{rb}",
                                                        name=f"t1{half}_{ht}_{rb}")
                                      nc.scalar.activation(t1t[:], pdt[:],
                                                           Act.Copy,
                                                           scale=sc["d1"][:])
                                      t1[rb] = t1t
                                  elif mat == "i":
                                      ob = ospool.tile([128, HT_W], F32,
                                                       tag=f"ob_{rb}",
                                                       name=f"ob{half}_{ht}_{rb}")
                                      nc.scalar.activation(ob[:], pdt[:],
                                                           Act.Copy,
                                                           scale=sc["d2"][:])
                                      o_r = opool.tile([128, HT_W], F32,
                                                       tag="o_r",
                                                       name=f"o_r{half}_{ht}_{rb}")
                                      nc.vector.tensor_add(out=o_r[:],
                                                           in0=t1[rb][:],
                                                           in1=ob[:])
                                      nc.sync.dma_start(
                                          or_d.ap()[rr:rr + 128, h0:h0 + hw],
                                          o_r[:])
                                      t2t = ospool.tile([128, HT_W], F32,
                                                        tag=f"t2_{rb}",
                                                        name=f"t2{half}_{ht}_{rb}")
                                      nc.vector.tensor_sub(out=t2t[:], in0=ob[:],
                                                           in1=t1[rb][:])
                                      t2[rb] = t2t
                                  else:
                                      o_i = opool.tile([128, HT_W], F32,
                                                       tag="o_i",
                                                       name=f"o_i{half}_{ht}_{rb}")
                                      nc.vector.tensor_tensor(
                                          out=o_i[:], in0=pdt[:], in1=t2[rb][:],
                                          op=Alu.add)
                                      nc.sync.dma_start(
                                          oi_d.ap()[rr:rr + 128, h0:h0 + hw],
                                          o_i[:])

    nc.compile()
    return nc


_CACHE = {}


def _get_program(key):
    if key not in _CACHE:
        _CACHE[key] = _build_program(*key)
    return _CACHE[key]


def kernel(x_real, x_imag, gate_wr, gate_wi, up_wr, up_wi,
           down_wr, down_wi, ln_wr, ln_wi, **run_kwargs):
    tgr, tgi, rm_g, im_g = _weight_prep(np.asarray(gate_wr), np.asarray(gate_wi))
    tur, tui, rm_u, im_u = _weight_prep(np.asarray(up_wr), np.asarray(up_wi))
    tdr, tdi, rm_d, im_d = _weight_prep(np.asarray(down_wr), np.asarray(down_wi))

    nc = _get_program((rm_g, im_g, rm_u, im_u, rm_d, im_d))

    wuc = tur * np.float32(rm_u) + tui * np.float32(im_u)
    wdc = tdr * np.float32(rm_d) + tdi * np.float32(im_d)

    shared = {
        "wgr_t": _to_bf16_T(tgr), "wgi_t": _to_bf16_T(tgi),
        "wur_t": _to_bf16_T(tur), "wui_t": _to_bf16_T(tui),
        "wuc_t": _to_bf16_T(wuc),
        "wdr_t": _to_bf16_T(tdr), "wdi_t": _to_bf16_T(tdi),
        "wdc_t": _to_bf16_T(wdc),
        "ln_r": np.asarray(ln_wr, np.float32).reshape(1, IM),
        "ln_i": np.asarray(ln_wi, np.float32).reshape(1, IM),
    }
    xr = np.ascontiguousarray(np.asarray(x_real, np.float32).reshape(ROWS, HIDDEN))
    xi = np.ascontiguousarray(np.asarray(x_imag, np.float32).reshape(ROWS, HIDDEN))

    in_maps = []
    for c in range(NCORES):
        sl = slice(c * RPC, (c + 1) * RPC)
        in_maps.append({"xr": np.ascontiguousarray(xr[sl]),
                        "xi": np.ascontiguousarray(xi[sl]), **shared})

    res = run_bass_kernel_spmd(nc, in_maps, core_ids=list(range(NCORES)),
                               **run_kwargs)

    out_r = np.concatenate([res.results[c]["o_r"] for c in range(NCORES)],
                           axis=0).reshape(B, S, HIDDEN)
    out_i = np.concatenate([res.results[c]["o_i"] for c in range(NCORES)],
                           axis=0).reshape(B, S, HIDDEN)
    kernel.last_results = res
    return out_r, out_i


# revision 7
# speedup vs baseline: 1.2691x; 1.2691x over previous
"""Trainium2 Bass kernel for nn_ComplexNetMLP (complex ternary-quant MLP).

Strategy: data-parallel over the 8192 rows across 8 NeuronCores (1024 rows
each, no collectives). The ternary weight quantization and int8 activation
fake-quant mean every matmul is (small int) x {-1,0,+1}: exactly
representable in bf16 with exact fp32 PSUM accumulation, so the big
matmuls run at bf16 rate with integer-exact results. Per-row / global
scales are applied on the vector engines afterwards.

v2: 3-multiplication (Karatsuba) complex matmul for the up and down
projections: P = qxr@qwr, Q = qxi@qwi, R = (qxr-qxi)@(qwr+qwi), then
out_r = P+Q (exact) and out_i = R-P+Q. The combined weight qwr+qwi has
entries {0, +-rm, +-im} (disjoint ternary support), stored in bf16; the
difference activation D = qxr-qxi is built on-device in bf16. This cuts
tensor-engine work from 4 to 3 matmul units per complex linear. The gate
projection keeps 4 exact matmuls because its output feeds the relu2 sign
threshold, where bf16-sized errors in g_i cause mask flips worth ~2%
output error.

Host side only reformats weights: ternary sign matrices (bf16, pre
transposed so the contraction dim lands on SBUF partitions), combined
Karatsuba matrices for up/down, and global scale scalars.
"""

import sys

sys.path.insert(0, "/opt/trn_rl_repo")

import numpy as np
import ml_dtypes

import concourse.bass as bass
import concourse.tile as tile
from concourse import bacc, mybir
from concourse.bass_utils import run_bass_kernel_spmd

F32 = mybir.dt.float32
BF16 = mybir.dt.bfloat16

HIDDEN = 2048
IM = 5504
B, S = 4, 2048
ROWS = B * S              # 8192
NCORES = 8
RPC = ROWS // NCORES      # 1024 rows per core
HROWS = RPC // 2          # 512 rows per half
NRB = HROWS // 128        # 4 row-blocks per half
KO = HIDDEN // 128        # 16 k-chunks for gate/up
IO = IM // 128            # 43 im-chunks for down
EPS = 1e-6
MAGIC = float(np.float32(12582912.0))  # 1.5 * 2**23, RNE rounding magic

# im tiles for phase 1 (gate/up output tiles)
P1_W = 256
IM_TILES = [(i * P1_W, min(P1_W, IM - i * P1_W)) for i in range((IM + P1_W - 1) // P1_W)]
HT_W = 256                 # hidden tile width for down proj
HID_TILES = [(i * HT_W, HT_W) for i in range(HIDDEN // HT_W)]
# phase-2 requant chunks: (io0, nio) covering IO=43
P2_CHUNKS = [(0, 22), (22, 21)]
P2_WMAX = 22 * 128

Alu = mybir.AluOpType
Act = mybir.ActivationFunctionType


def _weight_prep(wr, wi):
    """Replicate reference.weight_quant: ternary sign matrices + scale.

    Returns (tr, ti) sign matrices in {-1,0,1} and (rmean, imean)
    clipped scales so qwr = tr * rm, qwi = ti * im.
    Masks via |wr| vs |wi| (equivalent to the phase comparisons except on
    measure-zero boundaries).
    """
    awr = np.abs(wr)
    awi = np.abs(wi)
    rmask = awr > awi
    imask = ~rmask
    tr = np.where(rmask, np.sign(wr), 0.0).astype(np.float32)
    ti = np.where(imask, np.sign(wi), 0.0).astype(np.float32)
    rcnt = np.maximum(rmask.sum(dtype=np.float32), 1.0)
    icnt = np.maximum(imask.sum(dtype=np.float32), 1.0)
    rmean = np.float32(np.sum(awr * rmask, dtype=np.float32) / rcnt)
    imean = np.float32(np.sum(awi * imask, dtype=np.float32) / icnt)
    rm = float(np.clip(rmean, np.float32(1e-5), None))
    im = float(np.clip(imean, np.float32(1e-5), None))
    return tr, ti, rm, im


def _to_bf16_T(t):
    """Transpose and cast a weight matrix to contiguous bf16."""
    return np.ascontiguousarray(t.T).astype(ml_dtypes.bfloat16)


def _build_program(rm_g, im_g, rm_u, im_u, rm_d, im_d, rpc=RPC):
    """Trace the per-core Bass program (same program for all 8 cores)."""
    HROWS = rpc // 2
    NRB = HROWS // 128
    nc = bacc.Bacc("TRN2", target_bir_lowering=False, debug=False,
                   num_devices=NCORES)

    xr_d = nc.dram_tensor("xr", [rpc, HIDDEN], F32, kind="ExternalInput")
    xi_d = nc.dram_tensor("xi", [rpc, HIDDEN], F32, kind="ExternalInput")
    wgr_d = nc.dram_tensor("wgr_t", [HIDDEN, IM], BF16, kind="ExternalInput")
    wgi_d = nc.dram_tensor("wgi_t", [HIDDEN, IM], BF16, kind="ExternalInput")
    wur_d = nc.dram_tensor("wur_t", [HIDDEN, IM], BF16, kind="ExternalInput")
    wui_d = nc.dram_tensor("wui_t", [HIDDEN, IM], BF16, kind="ExternalInput")
    wuc_d = nc.dram_tensor("wuc_t", [HIDDEN, IM], BF16, kind="ExternalInput")
    wdr_d = nc.dram_tensor("wdr_t", [IM, HIDDEN], BF16, kind="ExternalInput")
    wdi_d = nc.dram_tensor("wdi_t", [IM, HIDDEN], BF16, kind="ExternalInput")
    wdc_d = nc.dram_tensor("wdc_t", [IM, HIDDEN], BF16, kind="ExternalInput")
    lnr_d = nc.dram_tensor("ln_r", [1, IM], F32, kind="ExternalInput")
    lni_d = nc.dram_tensor("ln_i", [1, IM], F32, kind="ExternalInput")
    or_d = nc.dram_tensor("o_r", [rpc, HIDDEN], F32, kind="ExternalOutput")
    oi_d = nc.dram_tensor("o_i", [rpc, HIDDEN], F32, kind="ExternalOutput")

    tr_d = nc.dram_tensor("t_r_scratch", [rpc, IM], F32)
    ti_d = nc.dram_tensor("t_i_scratch", [rpc, IM], F32)

    # weight DRAM views with the 128-partition k-chunk factored out
    wg_views = {
        "gr": wgr_d.ap().rearrange("(ko ki) n -> ki ko n", ki=128),
        "gi": wgi_d.ap().rearrange("(ko ki) n -> ki ko n", ki=128),
        "ur": wur_d.ap().rearrange("(ko ki) n -> ki ko n", ki=128),
        "ui": wui_d.ap().rearrange("(ko ki) n -> ki ko n", ki=128),
        "uc": wuc_d.ap().rearrange("(ko ki) n -> ki ko n", ki=128),
    }
    wd_views = {
        "dr": wdr_d.ap().rearrange("(io ii) n -> ii io n", ii=128),
        "di": wdi_d.ap().rearrange("(io ii) n -> ii io n", ii=128),
        "dc": wdc_d.ap().rearrange("(io ii) n -> ii io n", ii=128),
    }

    def ln_bcast(handle, n0, w):
        return bass.AP(tensor=handle, offset=n0, ap=[[0, 128], [1, w]])

    with tile.TileContext(nc) as tc:
        for half in range(2):
            r0 = half * HROWS

            with tc.tile_pool(name=f"stats{half}", bufs=1) as stats:
              with tc.tile_pool(name=f"xqt{half}", bufs=1) as xqt_pool:

                # ---- prologue: act quant of x + D = qxr - qxi + transposes ----
                xqt = {"r": [], "i": []}
                dxt = []                     # transposed D tiles per rb
                mclip = {"r": [], "i": []}   # clip(absmax, 1e-5) per rb
                alpha = []                   # per rb dict of matmul scales
                _sid = nc.enter_named_scope(f"prol{half}", False)[0]
                for rb in range(NRB):
                    rr = r0 + rb * 128
                    al = {}
                    with tc.tile_pool(name=f"prol{half}_{rb}", bufs=1) as pp:
                        xq = {}
                        for comp, x_d in (("r", xr_d), ("i", xi_d)):
                            xt = pp.tile([128, HIDDEN], F32,
                                         name=f"x{comp}{half}{rb}")
                            nc.scalar.dma_start(xt[:], x_d.ap()[rr:rr + 128, :])
                            am = stats.tile([128, 1], F32,
                                            name=f"am{comp}{half}{rb}")
                            nc.vector.tensor_reduce(
                                out=am[:], in_=xt[:], axis=mybir.AxisListType.X,
                                op=Alu.max, apply_absolute_value=True)
                            m = stats.tile([128, 1], F32,
                                           name=f"m{comp}{half}{rb}")
                            nc.vector.tensor_scalar_max(out=m[:], in0=am[:],
                                                        scalar1=1e-5)
                            mclip[comp].append(m)
                            rcp = stats.tile([128, 1], F32,
                                             name=f"rcp{comp}{half}{rb}")
                            nc.vector.reciprocal(rcp[:], m[:])
                            c = stats.tile([128, 1], F32,
                                           name=f"c{comp}{half}{rb}")
                            nc.vector.tensor_scalar_mul(out=c[:], in0=rcp[:],
                                                        scalar1=127.0)
                            s1 = pp.tile([128, HIDDEN], F32,
                                         name=f"s1{comp}{half}{rb}")
                            nc.vector.tensor_scalar(
                                out=s1[:], in0=xt[:], scalar1=c[:],
                                scalar2=MAGIC, op0=Alu.mult, op1=Alu.add)
                            xqc = pp.tile([128, HIDDEN], BF16,
                                          name=f"xq{comp}{half}{rb}")
                            nc.vector.tensor_scalar(
                                out=xqc[:], in0=s1[:], scalar1=MAGIC,
                                scalar2=None, op0=Alu.subtract, op1=Alu.bypass)
                            xq[comp] = xqc
                            xqt_t = xqt_pool.tile([128, KO, 128], BF16,
                                                  name=f"xqt{comp}{half}{rb}")
                            nc.sync.dma_start_transpose(xqt_t[:], xqc[:])
                            xqt[comp].append(xqt_t)
                        # D = qxr - qxi = xq_r*(mr/127) - xq_i*(mi/127)
                        sr5 = stats.tile([128, 1], F32, name=f"sr5{half}{rb}")
                        nc.vector.tensor_scalar_mul(
                            out=sr5[:], in0=mclip["r"][rb][:], scalar1=1.0 / 127.0)
                        si5n = stats.tile([128, 1], F32, name=f"si5n{half}{rb}")
                        nc.vector.tensor_scalar_mul(
                            out=si5n[:], in0=mclip["i"][rb][:], scalar1=-1.0 / 127.0)
                        dtmp = pp.tile([128, HIDDEN], F32,
                                       name=f"dtmp{half}{rb}")
                        nc.scalar.activation(dtmp[:], xq["r"][:], Act.Copy,
                                             scale=sr5[:])
                        dbf = pp.tile([128, HIDDEN], BF16,
                                      name=f"dbf{half}{rb}")
                        nc.vector.scalar_tensor_tensor(
                            out=dbf[:], in0=xq["i"][:], scalar=si5n[:],
                            in1=dtmp[:], op0=Alu.mult, op1=Alu.add)
                        dxt_t = xqt_pool.tile([128, KO, 128], BF16,
                                              name=f"dxt{half}{rb}")
                        nc.sync.dma_start_transpose(dxt_t[:], dbf[:])
                        dxt.append(dxt_t)
                    # per-row matmul output scales
                    for nm, const, m in (
                        ("g1", rm_g, mclip["r"][rb]), ("g2", im_g, mclip["i"][rb]),
                        ("g3", im_g, mclip["r"][rb]), ("g4", rm_g, mclip["i"][rb]),
                        ("u1", rm_u, mclip["r"][rb]), ("u2", im_u, mclip["i"][rb]),
                        ("u1n", -rm_u, mclip["r"][rb]),
                    ):
                        t = stats.tile([128, 1], F32, name=f"al{nm}{half}{rb}")
                        nc.vector.tensor_scalar_mul(out=t[:], in0=m[:],
                                                    scalar1=const / 127.0)
                        al[nm] = t
                    alpha.append(al)

                nc.leave_named_scope(f"prol{half}", _sid, False)
                # accumulators (ping-pong) per rb: sumsq r/i, absmax r/i
                acc = {k: [[stats.tile([128, 1], F32, name=f"acc{k}{half}{rb}{s}")
                            for s in range(2)] for rb in range(NRB)]
                       for k in ("ssr", "ssi", "amr", "ami")}

                # ---- phase 1: gate + up + relu2 + glu + stats ----
                with tc.tile_pool(name=f"wg{half}", bufs=2) as wpool, \
                     tc.tile_pool(name=f"ln{half}", bufs=2) as lnpool, \
                     tc.tile_pool(name=f"gsto{half}", bufs=2) as gpool, \
                     tc.tile_pool(name=f"tout{half}", bufs=3) as tpool, \
                     tc.tile_pool(name=f"scr{half}", bufs=2) as spool, \
                     tc.tile_pool(name=f"ps{half}", bufs=1, space="PSUM") as psp:
                    _sid = nc.enter_named_scope(f"p1_{half}", False)[0]
                    for it, (n0, w) in enumerate(IM_TILES):
                        wt = {}
                        for key in ("gr", "gi", "ur", "ui", "uc"):
                            t = wpool.tile([128, KO, P1_W], BF16, tag=f"w{key}",
                                           name=f"w{key}_{half}_{it}")
                            nc.sync.dma_start(t[:, :, :w],
                                              wg_views[key][:, :, n0:n0 + w])
                            wt[key] = t
                        lnr_t = lnpool.tile([128, P1_W], F32, tag="lnr",
                                            name=f"lnr{half}_{it}")
                        nc.gpsimd.dma_start(out=lnr_t[:, :w],
                                            in_=ln_bcast(lnr_d, n0, w))
                        lni_t = lnpool.tile([128, P1_W], F32, tag="lni",
                                            name=f"lni{half}_{it}")
                        nc.gpsimd.dma_start(out=lni_t[:, :w],
                                            in_=ln_bcast(lni_d, n0, w))

                        for rb in range(NRB):
                            rr = r0 + rb * 128
                            ps = {k: psp.tile([128, P1_W], F32, tag=f"ps{k}",
                                              name=f"ps{k}_{half}_{it}_{rb}")
                                  for k in ("g1", "g2", "g3", "g4",
                                            "u1", "u2", "uR")}
                            for ko in range(KO):
                                st = ko == 0
                                sp = ko == KO - 1
                                lr = xqt["r"][rb][:, ko, :]
                                li = xqt["i"][rb][:, ko, :]
                                ld = dxt[rb][:, ko, :]
                                nc.tensor.matmul(ps["g1"][:, :w], lr,
                                                 wt["gr"][:, ko, :w],
                                                 start=st, stop=sp)
                                nc.tensor.matmul(ps["g3"][:, :w], lr,
                                                 wt["gi"][:, ko, :w],
                                                 start=st, stop=sp)
                                nc.tensor.matmul(ps["u1"][:, :w], lr,
                                                 wt["ur"][:, ko, :w],
                                                 start=st, stop=sp)
                                nc.tensor.matmul(ps["g2"][:, :w], li,
                                                 wt["gi"][:, ko, :w],
                                                 start=st, stop=sp)
                                nc.tensor.matmul(ps["g4"][:, :w], li,
                                                 wt["gr"][:, ko, :w],
                                                 start=st, stop=sp)
                                nc.tensor.matmul(ps["u2"][:, :w], li,
                                                 wt["ui"][:, ko, :w],
                                                 start=st, stop=sp)
                                nc.tensor.matmul(ps["uR"][:, :w], ld,
                                                 wt["uc"][:, ko, :w],
                                                 start=st, stop=sp)

                            al = alpha[rb]
                            # g_r = a1*G1 + a2*G2 ; g_i = a3*G3 - a4*G4
                            tb = spool.tile([128, P1_W], F32, tag="tb",
                                            name=f"tb{half}_{it}_{rb}")
                            nc.scalar.activation(tb[:, :w], ps["g2"][:, :w],
                                                 Act.Copy, scale=al["g2"][:])
                            g_r = gpool.tile([128, P1_W], F32, tag="g_r",
                                             name=f"g_r{half}_{it}_{rb}")
                            nc.vector.scalar_tensor_tensor(
                                out=g_r[:, :w], in0=ps["g1"][:, :w],
                                scalar=al["g1"][:], in1=tb[:, :w],
                                op0=Alu.mult, op1=Alu.add)
                            tb2 = spool.tile([128, P1_W], F32, tag="tb2",
                                             name=f"tb2{half}_{it}_{rb}")
                            nc.scalar.activation(tb2[:, :w], ps["g4"][:, :w],
                                                 Act.Copy, scale=al["g4"][:])
                            g_i = gpool.tile([128, P1_W], F32, tag="g_i",
                                             name=f"g_i{half}_{it}_{rb}")
                            nc.vector.scalar_tensor_tensor(
                                out=g_i[:, :w], in0=ps["g3"][:, :w],
                                scalar=al["g3"][:], in1=tb2[:, :w],
                                op0=Alu.mult, op1=Alu.subtract)
                            # u_r = b1*U1 + b2*U2 ; u_i = R - b1*U1 + b2*U2
                            tb3 = spool.tile([128, P1_W], F32, tag="tb3",
                                             name=f"tb3{half}_{it}_{rb}")
                            nc.scalar.activation(tb3[:, :w], ps["u2"][:, :w],
                                                 Act.Copy, scale=al["u2"][:])
                            u_r = gpool.tile([128, P1_W], F32, tag="u_r",
                                             name=f"u_r{half}_{it}_{rb}")
                            nc.vector.scalar_tensor_tensor(
                                out=u_r[:, :w], in0=ps["u1"][:, :w],
                                scalar=al["u1"][:], in1=tb3[:, :w],
                                op0=Alu.mult, op1=Alu.add)
                            tb4 = spool.tile([128, P1_W], F32, tag="tb4",
                                             name=f"tb4{half}_{it}_{rb}")
                            nc.vector.tensor_tensor(
                                out=tb4[:, :w], in0=ps["uR"][:, :w],
                                in1=tb3[:, :w], op=Alu.add)
                            u_i = gpool.tile([128, P1_W], F32, tag="u_i",
                                             name=f"u_i{half}_{it}_{rb}")
                            nc.vector.scalar_tensor_tensor(
                                out=u_i[:, :w], in0=ps["u1"][:, :w],
                                scalar=al["u1n"][:], in1=tb4[:, :w],
                                op0=Alu.mult, op1=Alu.add)

                            # relu2 keep-mask c = (max(g_r, g_i) >= 0)
                            cm = spool.tile([128, P1_W], F32, tag="cm",
                                            name=f"cm{half}_{it}_{rb}")
                            nc.vector.tensor_max(out=cm[:, :w], in0=g_r[:, :w],
                                                 in1=g_i[:, :w])
                            nc.vector.tensor_scalar(
                                out=cm[:, :w], in0=cm[:, :w], scalar1=0.0,
                                scalar2=None, op0=Alu.is_ge, op1=Alu.bypass)
                            p2 = spool.tile([128, P1_W], F32, tag="p2",
                                            name=f"p2{half}_{it}_{rb}")
                            nc.scalar.activation(p2[:, :w], g_r[:, :w],
                                                 Act.Square)
                            q2 = spool.tile([128, P1_W], F32, tag="q2",
                                            name=f"q2{half}_{it}_{rb}")
                            nc.scalar.activation(q2[:, :w], g_i[:, :w],
                                                 Act.Square)
                            # h_r = (p2*u_r + q2*u_i) * c
                            t1 = spool.tile([128, P1_W], F32, tag="t1",
                                            name=f"t1{half}_{it}_{rb}")
                            nc.vector.tensor_mul(out=t1[:, :w], in0=p2[:, :w],
                                                 in1=u_r[:, :w])
                            t2 = spool.tile([128, P1_W], F32, tag="t2",
                                            name=f"t2{half}_{it}_{rb}")
                            nc.vector.tensor_mul(out=t2[:, :w], in0=q2[:, :w],
                                                 in1=u_i[:, :w])
                            h_r = spool.tile([128, P1_W], F32, tag="h_r",
                                             name=f"h_r{half}_{it}_{rb}")
                            nc.vector.tensor_add(out=h_r[:, :w], in0=t1[:, :w],
                                                 in1=t2[:, :w])
                            nc.vector.tensor_mul(out=h_r[:, :w], in0=h_r[:, :w],
                                                 in1=cm[:, :w])
                            # h_i = (p2*u_i - q2*u_r) * c
                            t3 = spool.tile([128, P1_W], F32, tag="t3",
                                            name=f"t3{half}_{it}_{rb}")
                            nc.vector.tensor_mul(out=t3[:, :w], in0=p2[:, :w],
                                                 in1=u_i[:, :w])
                            t4 = spool.tile([128, P1_W], F32, tag="t4",
                                            name=f"t4{half}_{it}_{rb}")
                            nc.vector.tensor_mul(out=t4[:, :w], in0=q2[:, :w],
                                                 in1=u_r[:, :w])
                            h_i = spool.tile([128, P1_W], F32, tag="h_i",
                                             name=f"h_i{half}_{it}_{rb}")
                            nc.vector.tensor_sub(out=h_i[:, :w], in0=t3[:, :w],
                                                 in1=t4[:, :w])
                            nc.vector.tensor_mul(out=h_i[:, :w], in0=h_i[:, :w],
                                                 in1=cm[:, :w])

                            # stats + ln-scaled spill tiles
                            sq = spool.tile([128, P1_W], F32, tag="sq",
                                            name=f"sq{half}_{it}_{rb}")
                            nc.vector.tensor_mul(out=sq[:, :w], in0=h_r[:, :w],
                                                 in1=h_r[:, :w])
                            ssp = spool.tile([128, 1], F32, tag="ssp",
                                             name=f"ssp{half}_{it}_{rb}")
                            nc.vector.tensor_reduce(
                                out=ssp[:], in_=sq[:, :w], axis=mybir.AxisListType.X,
                                op=Alu.add)
                            if it == 0:
                                nc.vector.tensor_copy(out=acc["ssr"][rb][0][:],
                                                      in_=ssp[:])
                            else:
                                nc.vector.tensor_add(
                                    out=acc["ssr"][rb][it % 2][:],
                                    in0=acc["ssr"][rb][(it - 1) % 2][:], in1=ssp[:])
                            t_r = tpool.tile([128, P1_W], F32, tag="t_r",
                                             name=f"t_r{half}_{it}_{rb}")
                            nc.vector.tensor_mul(out=t_r[:, :w], in0=h_r[:, :w],
                                                 in1=lnr_t[:, :w])
                            amp = spool.tile([128, 1], F32, tag="amp",
                                             name=f"amp{half}_{it}_{rb}")
                            nc.vector.tensor_reduce(
                                out=amp[:], in_=t_r[:, :w], axis=mybir.AxisListType.X,
                                op=Alu.max, apply_absolute_value=True)
                            if it == 0:
                                nc.vector.tensor_scalar_max(
                                    out=acc["amr"][rb][0][:], in0=amp[:], scalar1=0.0)
                            else:
                                nc.vector.tensor_tensor(
                                    out=acc["amr"][rb][it % 2][:],
                                    in0=acc["amr"][rb][(it - 1) % 2][:],
                                    in1=amp[:], op=Alu.max)
                            sq2 = spool.tile([128, P1_W], F32, tag="sq2",
                                             name=f"sq2{half}_{it}_{rb}")
                            nc.vector.tensor_mul(out=sq2[:, :w], in0=h_i[:, :w],
                                                 in1=h_i[:, :w])
                            ssp2 = spool.tile([128, 1], F32, tag="ssp2",
                                              name=f"ssp2{half}_{it}_{rb}")
                            nc.vector.tensor_reduce(
                                out=ssp2[:], in_=sq2[:, :w], axis=mybir.AxisListType.X,
                                op=Alu.add)
                            if it == 0:
                                nc.vector.tensor_copy(out=acc["ssi"][rb][0][:],
                                                      in_=ssp2[:])
                            else:
                                nc.vector.tensor_add(
                                    out=acc["ssi"][rb][it % 2][:],
                                    in0=acc["ssi"][rb][(it - 1) % 2][:], in1=ssp2[:])
                            t_i = tpool.tile([128, P1_W], F32, tag="t_i",
                                             name=f"t_i{half}_{it}_{rb}")
                            nc.vector.tensor_mul(out=t_i[:, :w], in0=h_i[:, :w],
                                                 in1=lni_t[:, :w])
                            amp2 = spool.tile([128, 1], F32, tag="amp2",
                                              name=f"amp2{half}_{it}_{rb}")
                            nc.vector.tensor_reduce(
                                out=amp2[:], in_=t_i[:, :w], axis=mybir.AxisListType.X,
                                op=Alu.max, apply_absolute_value=True)
                            if it == 0:
                                nc.vector.tensor_scalar_max(
                                    out=acc["ami"][rb][0][:], in0=amp2[:], scalar1=0.0)
                            else:
                                nc.vector.tensor_tensor(
                                    out=acc["ami"][rb][it % 2][:],
                                    in0=acc["ami"][rb][(it - 1) % 2][:],
                                    in1=amp2[:], op=Alu.max)
                            nc.gpsimd.dma_start(out=tr_d.ap()[rr:rr + 128, n0:n0 + w],
                                                in_=t_r[:, :w])
                            nc.gpsimd.dma_start(out=ti_d.ap()[rr:rr + 128, n0:n0 + w],
                                                in_=t_i[:, :w])

                    nc.leave_named_scope(f"p1_{half}", _sid, False)
              # ---- phase 2: rmsnorm scale + act quant of n + Dn + transposes ----
              last = (len(IM_TILES) - 1) % 2
              with tc.tile_pool(name=f"nqt{half}", bufs=1) as nqt_pool:
                  nqt = {"r": [], "i": []}
                  dnt = []
                  dsc = []
                  with tc.tile_pool(name=f"p2a{half}", bufs=2) as p2a, \
                       tc.tile_pool(name=f"p2s{half}", bufs=1) as p2s:
                      _sid = nc.enter_named_scope(f"p2_{half}", False)[0]
                      for rb in range(NRB):
                          rr = r0 + rb * 128
                          ssr = acc["ssr"][rb][last]
                          ssi = acc["ssi"][rb][last]
                          su = stats.tile([128, 1], F32, name=f"su{half}{rb}")
                          nc.vector.tensor_add(out=su[:], in0=ssr[:], in1=ssi[:])
                          me = stats.tile([128, 1], F32, name=f"me{half}{rb}")
                          nc.vector.tensor_scalar(
                              out=me[:], in0=su[:], scalar1=1.0 / IM,
                              scalar2=EPS, op0=Alu.mult, op1=Alu.add)
                          sr = stats.tile([128, 1], F32, name=f"sr{half}{rb}")
                          nc.scalar.activation(sr[:], me[:], Act.Sqrt)
                          inv0 = stats.tile([128, 1], F32, name=f"inv0{half}{rb}")
                          nc.vector.reciprocal(inv0[:], sr[:])
                          # one Newton step: inv = inv0*(1.5 - 0.5*me*inv0^2)
                          nw = stats.tile([128, 1], F32, name=f"nw{half}{rb}")
                          nc.vector.tensor_mul(out=nw[:], in0=inv0[:], in1=inv0[:])
                          nc.vector.tensor_mul(out=nw[:], in0=nw[:], in1=me[:])
                          nc.vector.tensor_scalar(
                              out=nw[:], in0=nw[:], scalar1=-0.5, scalar2=1.5,
                              op0=Alu.mult, op1=Alu.add)
                          inv = stats.tile([128, 1], F32, name=f"inv{half}{rb}")
                          nc.vector.tensor_mul(out=inv[:], in0=inv0[:], in1=nw[:])

                          sc = {}
                          for comp, amk in (("r", "amr"), ("i", "ami")):
                              am = acc[amk][rb][last]
                              amn = stats.tile([128, 1], F32,
                                               name=f"amn{comp}{half}{rb}")
                              nc.vector.tensor_mul(out=amn[:], in0=am[:], in1=inv[:])
                              nc.vector.tensor_scalar_max(out=amn[:], in0=amn[:],
                                                          scalar1=1e-5)
                              rsn = stats.tile([128, 1], F32,
                                               name=f"rsn{comp}{half}{rb}")
                              nc.vector.reciprocal(rsn[:], amn[:])
                              nc.vector.tensor_scalar_mul(out=rsn[:], in0=rsn[:],
                                                          scalar1=127.0)
                              cq = stats.tile([128, 1], F32,
                                              name=f"cq{comp}{half}{rb}")
                              nc.vector.tensor_mul(out=cq[:], in0=inv[:], in1=rsn[:])
                              sc[f"cq{comp}"] = cq
                              sc[f"amn{comp}"] = amn
                          # down-proj combine scales (Karatsuba)
                          for nm, const, amn in (
                              ("d1", rm_d, sc["amnr"]), ("d2", im_d, sc["amni"]),
                              ("d1n", -rm_d, sc["amnr"]),
                          ):
                              t = stats.tile([128, 1], F32,
                                             name=f"ds{nm}{half}{rb}")
                              nc.vector.tensor_scalar_mul(out=t[:], in0=amn[:],
                                                          scalar1=const / 127.0)
                              sc[nm] = t
                          # Dn = nq_r*(amnr/127) - nq_i*(amni/127)
                          cr5 = stats.tile([128, 1], F32, name=f"cr5{half}{rb}")
                          nc.vector.tensor_scalar_mul(
                              out=cr5[:], in0=sc["amnr"][:], scalar1=1.0 / 127.0)
                          ci5n = stats.tile([128, 1], F32, name=f"ci5n{half}{rb}")
                          nc.vector.tensor_scalar_mul(
                              out=ci5n[:], in0=sc["amni"][:], scalar1=-1.0 / 127.0)
                          dsc.append(sc)

                          nqt_t = {}
                          for comp in ("r", "i"):
                              nqt_t[comp] = nqt_pool.tile(
                                  [128, IO, 128], BF16, name=f"nqt{comp}{half}{rb}")
                              nqt[comp].append(nqt_t[comp])
                          dnt_t = nqt_pool.tile([128, IO, 128], BF16,
                                                name=f"dnt{half}{rb}")
                          dnt.append(dnt_t)

                          for ci, (io0, nio) in enumerate(P2_CHUNKS):
                              c0 = io0 * 128
                              w = nio * 128
                              nq = {}
                              for comp, t_d in (("r", tr_d), ("i", ti_d)):
                                  tin = p2a.tile([128, P2_WMAX], F32, tag="tin",
                                                 name=f"tin{comp}{half}{rb}{ci}")
                                  nc.scalar.dma_start(
                                      tin[:, :w], t_d.ap()[rr:rr + 128, c0:c0 + w])
                                  s1 = p2s.tile([128, P2_WMAX], F32, tag="s1q",
                                                name=f"s1q{comp}{half}{rb}{ci}")
                                  nc.scalar.activation(s1[:, :w], tin[:, :w],
                                                       Act.Copy, bias=MAGIC,
                                                       scale=sc[f"cq{comp}"][:])
                                  nqc = p2s.tile([128, P2_WMAX], BF16,
                                                 tag=f"nq{comp}",
                                                 name=f"nq{comp}{half}{rb}{ci}")
                                  nc.vector.tensor_scalar(
                                      out=nqc[:, :w], in0=s1[:, :w], scalar1=MAGIC,
                                      scalar2=None, op0=Alu.subtract, op1=Alu.bypass)
                                  nq[comp] = nqc
                                  nc.sync.dma_start_transpose(
                                      nqt_t[comp][:, io0:io0 + nio, :], nqc[:, :w])
                              dtm = p2s.tile([128, P2_WMAX], F32, tag="dtmp",
                                             name=f"dtm{half}{rb}{ci}")
                              nc.scalar.activation(dtm[:, :w], nq["r"][:, :w],
                                                   Act.Copy, scale=cr5[:])
                              dnc = p2s.tile([128, P2_WMAX], BF16, tag="dn",
                                             name=f"dnc{half}{rb}{ci}")
                              nc.vector.scalar_tensor_tensor(
                                  out=dnc[:, :w], in0=nq["i"][:, :w],
                                  scalar=ci5n[:], in1=dtm[:, :w],
                                  op0=Alu.mult, op1=Alu.add)
                              nc.sync.dma_start_transpose(
                                  dnt_t[:, io0:io0 + nio, :], dnc[:, :w])

                      nc.leave_named_scope(f"p2_{half}", _sid, False)
                  # ---- phase 3: down projection (Karatsuba) ----
                  with tc.tile_pool(name=f"dt{half}", bufs=2) as dtpool, \
                       tc.tile_pool(name=f"oo{half}", bufs=3) as opool, \
                       tc.tile_pool(name=f"od{half}", bufs=2) as ospool, \
                       tc.tile_pool(name=f"psd{half}", bufs=2,
                                    space="PSUM") as psd:
                      # Incremental combine keeps only one PSUM bank per rb
                      # live at a time:
                      #   after P:  t1 = d1*P            (pd bank freed)
                      #   after Q:  ob = d2*Q; o_r = t1+ob; t2 = ob-t1
                      #   after R:  o_i = R + t2
                      _sid = nc.enter_named_scope(f"p3_{half}", False)[0]
                      for ht, (h0, hw) in enumerate(HID_TILES):
                          t1 = {}
                          t2 = {}
                          for mat, key, stat in (("r", "dr", nqt["r"]),
                                                 ("i", "di", nqt["i"]),
                                                 ("c", "dc", dnt)):
                              wtd = dtpool.tile([128, IO, HT_W], BF16, tag="wd",
                                                name=f"wd{mat}{half}_{ht}")
                              nc.sync.dma_start(wtd[:, :, :],
                                                wd_views[key][:, :, h0:h0 + hw])
                              for rb in range(NRB):
                                  rr = r0 + rb * 128
                                  sc = dsc[rb]
                                  pdt = psd.tile([128, HT_W], F32,
                                                 tag=f"pd{rb}",
                                                 name=f"pd{mat}{rb}_{half}_{ht}")
                                  for io in range(IO):
                                      nc.tensor.matmul(pdt[:], stat[rb][:, io, :],
                                                       wtd[:, io, :],
                                                       start=io == 0,
                                                       stop=io == IO - 1)
                                  if mat == "r":
                                      t1t = ospool.tile([128, HT_W], F32,
                                                        tag=f"t1_{rb}",
                                                        name=f"t1{half}_{ht}_{rb}")
                                      nc.scalar.activation(t1t[:], pdt[:],
                                                           Act.Copy,
                                                           scale=sc["d1"][:])
                                      t1[rb] = t1t
                                  elif mat == "i":
                                      ob = ospool.tile([128, HT_W], F32,
                                                       tag=f"ob_{rb}",
                                                       name=f"ob{half}_{ht}_{rb}")
                                      nc.scalar.activation(ob[:], pdt[:],
                                                           Act.Copy,
                                                           scale=sc["d2"][:])
                                      o_r = opool.tile([128, HT_W], F32,
                                                       tag="o_r",
                                                       name=f"o_r{half}_{ht}_{rb}")
                                      nc.vector.tensor_add(out=o_r[:],
                                                           in0=t1[rb][:],
                                                           in1=ob[:])
                                      nc.gpsimd.dma_start(
                                          out=or_d.ap()[rr:rr + 128, h0:h0 + hw],
                                          in_=o_r[:])
                                      t2t = ospool.tile([128, HT_W], F32,
                                                        tag=f"t2_{rb}",
                                                        name=f"t2{half}_{ht}_{rb}")
                                      nc.vector.tensor_sub(out=t2t[:], in0=ob[:],
                                                           in1=t1[rb][:])
                                      t2[rb] = t2t
                                  else:
                                      o_i = opool.tile([128, HT_W], F32,
                                                       tag="o_i",
                                                       name=f"o_i{half}_{ht}_{rb}")
                                      nc.vector.tensor_tensor(
                                          out=o_i[:], in0=pdt[:], in1=t2[rb][:],
                                          op=Alu.add)
                                      nc.gpsimd.dma_start(
                                          out=oi_d.ap()[rr:rr + 128, h0:h0 + hw],
                                          in_=o_i[:])
                      nc.leave_named_scope(f"p3_{half}", _sid, False)

    nc.compile()
    return nc


_CACHE = {}


def _get_program(key):
    if key not in _CACHE:
        _CACHE[key] = _build_program(*key)
    return _CACHE[key]


def kernel(x_real, x_imag, gate_wr, gate_wi, up_wr, up_wi,
           down_wr, down_wi, ln_wr, ln_wi, **run_kwargs):
    tgr, tgi, rm_g, im_g = _weight_prep(np.asarray(gate_wr), np.asarray(gate_wi))
    tur, tui, rm_u, im_u = _weight_prep(np.asarray(up_wr), np.asarray(up_wi))
    tdr, tdi, rm_d, im_d = _weight_prep(np.asarray(down_wr), np.asarray(down_wi))

    nc = _get_program((rm_g, im_g, rm_u, im_u, rm_d, im_d))

    wuc = tur * np.float32(rm_u) + tui * np.float32(im_u)
    wdc = tdr * np.float32(rm_d) + tdi * np.float32(im_d)

    shared = {
        "wgr_t": _to_bf16_T(tgr), "wgi_t": _to_bf16_T(tgi),
        "wur_t": _to_bf16_T(tur), "wui_t": _to_bf16_T(tui),
        "wuc_t": _to_bf16_T(wuc),
        "wdr_t": _to_bf16_T(tdr), "wdi_t": _to_bf16_T(tdi),
        "wdc_t": _to_bf16_T(wdc),
        "ln_r": np.asarray(ln_wr, np.float32).reshape(1, IM),
        "ln_i": np.asarray(ln_wi, np.float32).reshape(1, IM),
    }
    xr = np.ascontiguousarray(np.asarray(x_real, np.float32).reshape(ROWS, HIDDEN))
    xi = np.ascontiguousarray(np.asarray(x_imag, np.float32).reshape(ROWS, HIDDEN))

    in_maps = []
    for c in range(NCORES):
        sl = slice(c * RPC, (c + 1) * RPC)
        in_maps.append({"xr": np.ascontiguousarray(xr[sl]),
                        "xi": np.ascontiguousarray(xi[sl]), **shared})

    res = run_bass_kernel_spmd(nc, in_maps, core_ids=list(range(NCORES)),
                               **run_kwargs)

    out_r = np.concatenate([res.results[c]["o_r"] for c in range(NCORES)],
                           axis=0).reshape(B, S, HIDDEN)
    out_i = np.concatenate([res.results[c]["o_i"] for c in range(NCORES)],
                           axis=0).reshape(B, S, HIDDEN)
    kernel.last_results = res
    return out_r, out_i
